# revision 23
# baseline (speedup 1.0000x reference)
"""BiLevelRoutingAttention Trainium2 kernel (v5).

Strategy (8 NeuronCores, data-parallel over batch: 2 batches/core, 32 (b,t)
tiles per core):
  - Host: transpose x to feature-major bf16; ROUTING ON HOST (fp64 window
    sums -> region features -> sim -> top-4 -> additive window mask), mask
    uploaded pre-expanded as [8,128] bf16 matmul lhsT slices that a matmul
    against static one-hot e8r rows expands onto the scores inside PSUM.
  - Device, per (b,t) tile, all layouts feature-major ("T-layout"):
      qT/kT = W^T x^T (bf16 matmuls, fp32 PSUM), V token-major.
      scoresT + mask accumulated in PSUM, exp on ACT (scale folded),
      Z via ones[128,32]-matmuls -> Z broadcast in PSUM (no DRAM bounce),
      reciprocal_approx_fast (DVE), PV col-packed, normalize, out
      projection, store fp32.
  - Emission is a fine-grained software pipeline: per-engine queues are
    in-order, so stages of adjacent tiles are interleaved such that the
    PE always has data-ready matmuls queued while ACT runs exp, and no
    engine's queue head ever waits on work emitted later (deadlock-free
    by construction; PSUM bank sharing pairs chosen to keep all pool
    reuse waits pointing backwards in emission order).
"""

import sys

sys.path.insert(0, "/opt/trn_rl_repo")

import numpy as np
import ml_dtypes

import concourse.bass as bass
import concourse.bacc as bacc
import concourse.mybir as mybir
import concourse.tile as tile
from concourse.bass_utils import run_bass_kernel_spmd

BF16 = mybir.dt.bfloat16
F32 = mybir.dt.float32

NCORES = 8
B, T, S, C = 16, 16, 256, 256
NW, WIN, NH, D, TK = 8, 32, 8, 32, 4
BPC = B // NCORES  # batches per core
SCALE = float(D) ** -0.5
MASKVAL = -1e9

_CACHE = {}


class _Kern:
    """Stage emitters for one (b, t) tile; handles live on self.h[i]."""

    def __init__(self, nc, pools, consts, tiles, has_bqk, has_bf):
        self.nc = nc
        (self.xp, self.mp, self.ep, self.qz, self.vp, self.pat,
         self.psc) = pools
        self.c = consts
        self.tiles = tiles
        self.h = [dict() for _ in tiles]
        self.has_bqk = has_bqk
        self.has_bf = has_bf

    # -- A: loads + qkv projections + PSUM->SBUF casts ------------------
    def stA(self, i):
        nc, c, h = self.nc, self.c, self.h[i]
        b, t = self.tiles[i]
        AL = mybir.AluOpType
        xt_sb = self.xp.tile([128, 2, S], BF16, tag="xt", name="xt_sb")
        nc.sync.dma_start(
            out=xt_sb,
            in_=c["xt_d"][b, t].rearrange("(cc p) s -> p cc s", p=128))
        mw_sb = self.xp.tile([128, 2, 128], BF16, tag="mw", name="mw_sb")
        nc.sync.dma_start(
            out=mw_sb[:].rearrange("p a k -> p (a k)"),
            in_=c["mw_d"][b, t].rearrange("p a k -> p (a k)"))
        h["mw"] = mw_sb

        qk_sb = self.mp.tile([128, 4, S], BF16, tag="qk", name="qk_sb")
        for half in range(2):
            qps = self.qz.tile([128, 2, S], F32, tag="qz", name="qps")
            for j in range(2):
                jb = 2 * half + j
                for cc in range(2):
                    nc.tensor.matmul(
                        qps[:, j, :],
                        lhsT=c["wqk"][:, cc, jb * 128:(jb + 1) * 128],
                        rhs=xt_sb[:, cc, :],
                        start=(j == 0 and cc == 0),
                        stop=(j == 1 and cc == 1))
            if self.has_bqk:
                nc.vector.tensor_tensor(
                    out=qk_sb[:, 2 * half:2 * half + 2, :], in0=qps,
                    in1=c["bqk"][:, 2 * half:2 * half + 2].unsqueeze(-1)
                        .to_broadcast([128, 2, S]),
                    op=AL.add)
            else:
                nc.vector.tensor_copy(
                    out=qk_sb[:, 2 * half:2 * half + 2, :], in_=qps)
        h["qk"] = qk_sb

        v_sb = self.mp.tile([128, 2, C], BF16, tag="v", name="v_sb")
        vps = self.vp.tile([128, 2, C], F32, tag="vp", name="vps")
        for sb_ in range(2):
            for cc in range(2):
                nc.tensor.matmul(vps[:, sb_, :],
                                 lhsT=xt_sb[:, cc, sb_ * 128:(sb_ + 1) * 128],
                                 rhs=c["wv"][:, cc, :],
                                 start=(sb_ == 0 and cc == 0),
                                 stop=(sb_ == 1 and cc == 1))
        nc.vector.tensor_copy(out=v_sb, in_=vps)
        h["v"] = v_sb

    # -- scores + mask for one (jbq, rg-pair) unit, then exp ------------
    # 2-bank PSUM tiles, bufs=2: while ACT exps one buffer the PE fills
    # the other; each concurrent row-group owns a full PSUM bank.
    def stSC(self, i, jbq, rgp):
        nc, c, h = self.nc, self.c, self.h[i]
        if jbq == 0 and rgp == 0:
            h["expT"] = self.ep.tile([128, 2, 4, 2 * S], BF16, tag="expT",
                                     name="expT")
        qk_sb, mw_sb = h["qk"], h["mw"]
        sc_ps = self.psc.tile([128, 2, 2 * S], F32, tag="sc", name="sc_ps")
        for rr in range(2):
            rg = 2 * rgp + rr
            nc.tensor.matmul(
                sc_ps[:, rr, :],
                lhsT=mw_sb[32 * rg:32 * rg + 16, jbq, :],
                rhs=c["e16r"][32 * rg:32 * rg + 16, :],
                start=True, stop=False,
                skip_group_check=True, tile_position=(32 * rg, 0))
            for kb in range(2):
                nc.tensor.matmul(
                    sc_ps[:, rr, kb * S:(kb + 1) * S],
                    lhsT=qk_sb[32 * rg:32 * rg + 32, 2 + jbq,
                               kb * 128:(kb + 1) * 128],
                    rhs=qk_sb[32 * rg:32 * rg + 32, jbq, :],
                    start=False, stop=(kb == 1),
                    skip_group_check=True, tile_position=(32 * rg, 0))
        nc.scalar.activation(
            out=h["expT"][:, jbq, 2 * rgp:2 * rgp + 2, :], in_=sc_ps,
            func=mybir.ActivationFunctionType.Exp, scale=SCALE)

    # -- PV matmuls for one jbq half; Z for both (in stZPV2) ------------
    def stPV(self, i, jbq):
        nc, c, h = self.nc, self.c, self.h[i]
        expT = h["expT"]
        if jbq == 0:
            h["at"] = self.pat.tile([128, 2, S], F32, tag="at", name="at")
        at = h["at"]
        for rg in range(4):
            hh = 4 * jbq + rg
            for kb in range(2):
                nc.tensor.matmul(at[32 * rg:32 * rg + 32, jbq, :],
                                 lhsT=h["v"][:, kb, 32 * hh:32 * hh + 32],
                                 rhs=expT[:, jbq, rg, kb * S:(kb + 1) * S],
                                 start=(jbq == 0 and kb == 0),
                                 stop=(jbq == 1 and kb == 1),
                                 skip_group_check=True,
                                 tile_position=(0, 32 * rg))

    def stZPV2(self, i):
        nc, c, h = self.nc, self.c, self.h[i]
        expT = h["expT"]
        h["zp"] = self.qz.tile([128, 2, S], F32, tag="qz", name="zp")
        zp = h["zp"]
        for rg in range(4):
            for kb in range(2):
                nc.tensor.matmul(
                    zp[32 * rg:32 * rg + 32, :, :],
                    lhsT=c["ones32"],
                    rhs=expT[:, :, rg, kb * S:(kb + 1) * S],
                    start=(kb == 0), stop=(kb == 1),
                    skip_group_check=True,
                    tile_position=(0, 32 * rg))
        self.stPV(i, 1)

    # -- E tail: reciprocal + normalize ---------------------------------
    def stNorm(self, i):
        nc, h = self.nc, self.h[i]
        AL = mybir.AluOpType
        rf_sb = self.mp.tile([128, 2, S], F32, tag="rf", name="rf_sb")
        nc.vector.reciprocal_approx_fast(out=rf_sb, in_=h["zp"])
        atn_sb = self.mp.tile([128, 2, S], BF16, tag="atn", name="atn_sb")
        nc.vector.tensor_tensor(out=atn_sb, in0=h["at"], in1=rf_sb,
                                op=AL.mult)
        h["atn"] = atn_sb

    # -- F: out projection + store --------------------------------------
    def stF(self, i):
        nc, c, h = self.nc, self.c, self.h[i]
        b, t = self.tiles[i]
        atn_sb = h["atn"]
        out_sb = self.mp.tile([128, 2, C], F32, tag="out", name="out_sb")
        po = self.vp.tile([128, 2, C], F32, tag="vp", name="po")
        for sb_ in range(2):
            for cc in range(2):
                nc.tensor.matmul(
                    po[:, sb_, :],
                    lhsT=atn_sb[:, cc, sb_ * 128:(sb_ + 1) * 128],
                    rhs=c["wp"][:, cc, :],
                    start=(sb_ == 0 and cc == 0),
                    stop=(not self.has_bf and sb_ == 1 and cc == 1))
            if self.has_bf:
                nc.tensor.matmul(po[:, sb_, :], lhsT=c["onesr"],
                                 rhs=c["bf"], start=False, stop=(sb_ == 1))
        nc.vector.tensor_copy(out=out_sb, in_=po)
        nc.sync.dma_start(out=c["out_d"][b, t].rearrange("s p c -> p s c"),
                          in_=out_sb)
        self.h[i] = {}  # drop handles


def _build_nc(has_bqk, has_bf, nt=T):
    nc = bacc.Bacc("TRN2", target_bir_lowering=False, debug=False)

    xt_d = nc.dram_tensor("xt", [BPC, nt, C, S], BF16, kind="ExternalInput")
    mw_d = nc.dram_tensor("mw16", [BPC, nt, 128, 2, 128], BF16,
                          kind="ExternalInput")
    wqk_d = nc.dram_tensor("wqk_bf", [C, 2 * C], BF16, kind="ExternalInput")
    wv_d = nc.dram_tensor("wv_bf", [C, C], BF16, kind="ExternalInput")
    wp_d = nc.dram_tensor("wproj_bf", [C, C], BF16, kind="ExternalInput")
    e8_d = nc.dram_tensor("e16r", [128, 2 * S], BF16, kind="ExternalInput")
    bqk_d = nc.dram_tensor("bqk_cols", [128, 4], F32, kind="ExternalInput")
    bf_d = nc.dram_tensor("bfinal_row", [1, C], BF16, kind="ExternalInput")
    out_d = nc.dram_tensor("out", [BPC, nt, 2, 128, C], F32,
                           kind="ExternalOutput")

    with tile.TileContext(nc) as tc:
        with (
            tc.tile_pool(name="wpool", bufs=1) as wp,
            tc.tile_pool(name="xpool", bufs=6) as xp,
            tc.tile_pool(name="mid", bufs=4) as mp,
            tc.tile_pool(name="exps", bufs=4) as ep,
            tc.tile_pool(name="qz", bufs=2, space="PSUM") as qz,
            tc.tile_pool(name="vp", bufs=1, space="PSUM") as vp,
            tc.tile_pool(name="at", bufs=1, space="PSUM") as pat,
            tc.tile_pool(name="sc", bufs=2, space="PSUM") as psc,
        ):
            consts = {"xt_d": xt_d, "mw_d": mw_d, "out_d": out_d}
            wqk_sb = wp.tile([128, 2, 2 * C], BF16)
            nc.sync.dma_start(out=wqk_sb,
                              in_=wqk_d.ap().rearrange("(cc p) j -> p cc j",
                                                       p=128))
            consts["wqk"] = wqk_sb
            wv_sb = wp.tile([128, 2, C], BF16)
            nc.sync.dma_start(out=wv_sb,
                              in_=wv_d.ap().rearrange("(cc p) j -> p cc j",
                                                      p=128))
            consts["wv"] = wv_sb
            wp_sb = wp.tile([128, 2, C], BF16)
            nc.sync.dma_start(out=wp_sb,
                              in_=wp_d.ap().rearrange("(cc p) j -> p cc j",
                                                      p=128))
            consts["wp"] = wp_sb
            e16_sb = wp.tile([128, 2 * S], BF16)
            nc.sync.dma_start(out=e16_sb, in_=e8_d.ap())
            consts["e16r"] = e16_sb
            ones32_sb = wp.tile([128, 32], BF16)
            nc.vector.memset(ones32_sb, 1.0)
            consts["ones32"] = ones32_sb
            # dummy exp: pulls the ~2.7us ACT table load to time zero so it
            # overlaps the initial weight/x DMAs instead of the first tile
            warm_sb = wp.tile([128, 1], F32)
            nc.vector.memset(warm_sb, 0.0)
            nc.scalar.activation(out=warm_sb, in_=warm_sb,
                                 func=mybir.ActivationFunctionType.Exp)
            if has_bqk:
                bqk_sb = wp.tile([128, 4], F32)
                nc.sync.dma_start(out=bqk_sb, in_=bqk_d.ap())
                consts["bqk"] = bqk_sb
            if has_bf:
                bf_sb = wp.tile([1, C], BF16)
                nc.sync.dma_start(out=bf_sb, in_=bf_d.ap())
                consts["bf"] = bf_sb
                onesr_sb = wp.tile([1, 128], BF16)
                nc.vector.memset(onesr_sb, 1.0)
                consts["onesr"] = onesr_sb

            tiles = [(b, t) for b in range(BPC) for t in range(nt)]
            K = _Kern(nc, (xp, mp, ep, qz, vp, pat, psc), consts, tiles,
                      has_bqk, has_bf)
            N = len(tiles)
            # Interleaved schedule: while ACT runs exp(i,0) the PE runs
            # Z/PV-jbq1(i-1) and qkv(i+1); while exp(i,1) runs the PE
            # runs Z/PV-jbq0(i) and proj(i-1).
            K.stA(0)
            for i in range(N):
                K.stSC(i, 0, 0)
                if i > 0:
                    K.stZPV2(i - 1)   # PV-jbq1(i-1) + Z(i-1)
                    K.stNorm(i - 1)
                K.stSC(i, 0, 1)
                if i + 1 < N:
                    K.stA(i + 1)
                K.stSC(i, 1, 0)
                K.stPV(i, 0)
                K.stSC(i, 1, 1)
                if i > 0:
                    K.stF(i - 1)
            K.stZPV2(N - 1)
            K.stNorm(N - 1)
            K.stF(N - 1)

    nc.compile()
    return nc


def _host_routing(x4, w_qkv, b_qkv):
    """Top-4 window routing in fp64 on host -> additive mask lhsT layout
    [B, T, 128, 2, 2, 128] bf16 (rows 32*rg + w = mask of q-window w,
    head 4*jbq+rg; free dims (jbq, kb); cols = keys of chunk kb)."""
    xsum = x4.reshape(B, T, NW, WIN, C).sum(3, dtype=np.float64)  # [B,T,NW,C]
    wq = w_qkv[:, :C].astype(np.float64)
    wk = w_qkv[:, C:2 * C].astype(np.float64)
    q_reg = xsum @ wq + WIN * b_qkv[:C].astype(np.float64)
    k_reg = xsum @ wk + WIN * b_qkv[C:2 * C].astype(np.float64)
    q_reg = q_reg.reshape(B, T, NW, NH, D)
    k_reg = k_reg.reshape(B, T, NW, NH, D)
    sim = np.einsum('btnhd,btmhd->bthnm', q_reg, k_reg)  # [B,T,h,NW,NW]
    thr = -np.partition(-sim, TK - 1, axis=-1)[..., TK - 1:TK]
    am = np.where(sim >= thr, 0.0, MASKVAL).astype(np.float32)
    nsel = (am == 0.0).sum(-1)
    if np.any(nsel != TK):  # pragma: no cover - ties are measure-zero
        idx = np.argsort(-sim, axis=-1, kind='stable')[..., :TK]
        am = np.full(sim.shape, MASKVAL, np.float32)
        np.put_along_axis(am, idx, 0.0, axis=-1)
    amx = np.repeat(am, WIN, axis=-1)  # [B,T,h,qw,256]
    mw16 = np.zeros((B, T, 128, 2, 128), np.float32)
    for jbq in range(2):
        for rg in range(4):
            hh = 4 * jbq + rg
            for kb in range(2):
                r0 = 32 * rg + 8 * kb
                mw16[:, :, r0:r0 + 8, jbq, :] = \
                    amx[:, :, hh, :, kb * 128:(kb + 1) * 128]
    return mw16.astype(ml_dtypes.bfloat16)


def _make_e16r():
    e = np.zeros((128, 2 * S), ml_dtypes.bfloat16)
    q = np.arange(S) // WIN  # query window of column q
    for rg in range(4):
        for kb in range(2):
            for w in range(NW):
                e[32 * rg + 8 * kb + w, kb * S + np.arange(S)[q == w]] = 1.0
    return e


def _host_prep(x, w_qkv, b_qkv, w_proj, b_proj):
    bf16 = ml_dtypes.bfloat16
    x4 = x.reshape(B, T, S, C)
    xt = np.ascontiguousarray(x4.transpose(0, 1, 3, 2)).astype(bf16)
    mw16 = _host_routing(x4, w_qkv, b_qkv)

    bfinal = b_qkv[2 * C:] @ w_proj + b_proj
    shared = {
        "wqk_bf": np.ascontiguousarray(w_qkv[:, :2 * C]).astype(bf16),
        "wv_bf": np.ascontiguousarray(w_qkv[:, 2 * C:]).astype(bf16),
        "wproj_bf": w_proj.astype(bf16),
        "e16r": _make_e16r(),
        "bqk_cols": np.ascontiguousarray(
            b_qkv[:2 * C].reshape(4, 128).T).astype(np.float32),
        "bfinal_row": bfinal.reshape(1, C).astype(bf16),
    }
    in_maps = []
    for core in range(NCORES):
        b0 = core * BPC
        m = dict(shared)
        m["xt"] = np.ascontiguousarray(xt[b0:b0 + BPC])
        m["mw16"] = np.ascontiguousarray(mw16[b0:b0 + BPC])
        in_maps.append(m)
    return in_maps


def kernel(x, w_qkv, b_qkv, w_proj, b_proj, **_unused_scalars):
    x = np.asarray(x, dtype=np.float32)
    w_qkv = np.asarray(w_qkv, dtype=np.float32)
    b_qkv = np.asarray(b_qkv, dtype=np.float32)
    w_proj = np.asarray(w_proj, dtype=np.float32)
    b_proj = np.asarray(b_proj, dtype=np.float32)

    has_bqk = bool(np.any(b_qkv[:2 * C]))
    bfinal = b_qkv[2 * C:] @ w_proj + b_proj
    has_bf = bool(np.any(bfinal))
    key = ("nc", has_bqk, has_bf)
    if key not in _CACHE:
        _CACHE[key] = _build_nc(has_bqk, has_bf)
        _CACHE["nc"] = _CACHE[key]
    nc = _CACHE[key]

    in_maps = _host_prep(x, w_qkv, b_qkv, w_proj, b_proj)
    res = run_bass_kernel_spmd(nc, in_maps, core_ids=list(range(NCORES)))

    out = np.empty((B, T, 2, 128, C), np.float32)
    for core in range(NCORES):
        out[core * BPC:(core + 1) * BPC] = res.results[core]["out"]
    # [B, T, sb, p, C] -> [B, T*S, C]
    return out.reshape(B, T * S, C)


# revision 25
# speedup vs baseline: 1.2068x; 1.2068x over previous
"""BiLevelRoutingAttention Trainium2 kernel (v5).

Strategy (8 NeuronCores, data-parallel over batch: 2 batches/core, 32 (b,t)
tiles per core):
  - Host: transpose x to feature-major bf16; ROUTING ON HOST (fp64 window
    sums -> region features -> sim -> top-4 -> additive window mask), mask
    uploaded pre-expanded as [8,128] bf16 matmul lhsT slices that a matmul
    against static one-hot e8r rows expands onto the scores inside PSUM.
  - Device, per (b,t) tile, all layouts feature-major ("T-layout"):
      qT/kT = W^T x^T (bf16 matmuls, fp32 PSUM), V token-major.
      scoresT + mask accumulated in PSUM, exp on ACT (scale folded),
      Z via ones[128,32]-matmuls -> Z broadcast in PSUM (no DRAM bounce),
      reciprocal_approx_fast (DVE), PV col-packed, normalize, out
      projection, store fp32.
  - Emission is a fine-grained software pipeline: per-engine queues are
    in-order, so stages of adjacent tiles are interleaved such that the
    PE always has data-ready matmuls queued while ACT runs exp, and no
    engine's queue head ever waits on work emitted later (deadlock-free
    by construction; PSUM bank sharing pairs chosen to keep all pool
    reuse waits pointing backwards in emission order).
"""

import sys

sys.path.insert(0, "/opt/trn_rl_repo")

import numpy as np
import ml_dtypes

import concourse.bass as bass
import concourse.bacc as bacc
import concourse.mybir as mybir
import concourse.tile as tile
from concourse.bass_utils import run_bass_kernel_spmd

BF16 = mybir.dt.bfloat16
F32 = mybir.dt.float32

NCORES = 8
B, T, S, C = 16, 16, 256, 256
NW, WIN, NH, D, TK = 8, 32, 8, 32, 4
BPC = B // NCORES  # batches per core
SCALE = float(D) ** -0.5
MASKVAL = -1e9

_CACHE = {}


class _Kern:
    """Stage emitters for one (b, t) tile; handles live on self.h[i]."""

    def __init__(self, nc, pools, consts, tiles, has_bqk, has_bf):
        self.nc = nc
        (self.xp, self.mp, self.ep, self.qz, self.vp, self.pat,
         self.psc) = pools
        self.c = consts
        self.tiles = tiles
        self.h = [dict() for _ in tiles]
        self.has_bqk = has_bqk
        self.has_bf = has_bf

    # -- A: loads + qkv projections + PSUM->SBUF casts ------------------
    def stA(self, i):
        nc, c, h = self.nc, self.c, self.h[i]
        b, t = self.tiles[i]
        AL = mybir.AluOpType
        xt_sb = self.xp.tile([128, 2, S], BF16, tag="xt", name="xt_sb")
        nc.sync.dma_start(
            out=xt_sb,
            in_=c["xt_d"][b, t].rearrange("(cc p) s -> p cc s", p=128))
        mw_sb = self.xp.tile([128, 2, 128], BF16, tag="mw", name="mw_sb")
        nc.sync.dma_start(
            out=mw_sb[:].rearrange("p a k -> p (a k)"),
            in_=c["mw_d"][b, t].rearrange("p a k -> p (a k)"))
        h["mw"] = mw_sb

        qk_sb = self.mp.tile([128, 4, S], BF16, tag="qk", name="qk_sb")
        for half in range(2):
            qps = self.qz.tile([128, 2, S], F32, tag="qz", name="qps")
            for j in range(2):
                jb = 2 * half + j
                for cc in range(2):
                    nc.tensor.matmul(
                        qps[:, j, :],
                        lhsT=c["wqk"][:, cc, jb * 128:(jb + 1) * 128],
                        rhs=xt_sb[:, cc, :],
                        start=(j == 0 and cc == 0),
                        stop=(j == 1 and cc == 1))
            if self.has_bqk:
                nc.vector.tensor_tensor(
                    out=qk_sb[:, 2 * half:2 * half + 2, :], in0=qps,
                    in1=c["bqk"][:, 2 * half:2 * half + 2].unsqueeze(-1)
                        .to_broadcast([128, 2, S]),
                    op=AL.add)
            else:
                nc.vector.tensor_copy(
                    out=qk_sb[:, 2 * half:2 * half + 2, :], in_=qps)
        h["qk"] = qk_sb

        v_sb = self.mp.tile([128, 2, C], BF16, tag="v", name="v_sb")
        vps = self.vp.tile([128, 2, C], F32, tag="vp", name="vps")
        for sb_ in range(2):
            for cc in range(2):
                nc.tensor.matmul(vps[:, sb_, :],
                                 lhsT=xt_sb[:, cc, sb_ * 128:(sb_ + 1) * 128],
                                 rhs=c["wv"][:, cc, :],
                                 start=(sb_ == 0 and cc == 0),
                                 stop=(sb_ == 1 and cc == 1))
        nc.vector.tensor_copy(out=v_sb, in_=vps)
        h["v"] = v_sb

    # -- scores + mask for one (jbq, rg-pair) unit, then exp ------------
    # 2-bank PSUM tiles, bufs=2: while ACT exps one buffer the PE fills
    # the other; each concurrent row-group owns a full PSUM bank.
    def stSC(self, i, jbq, rgp):
        nc, c, h = self.nc, self.c, self.h[i]
        if jbq == 0 and rgp == 0:
            h["expT"] = self.ep.tile([128, 2, 4, 2 * S], BF16, tag="expT",
                                     name="expT")
        qk_sb, mw_sb = h["qk"], h["mw"]
        sc_ps = self.psc.tile([128, 2, 2 * S], F32, tag="sc", name="sc_ps")
        for rr in range(2):
            rg = 2 * rgp + rr
            nc.tensor.matmul(
                sc_ps[:, rr, :],
                lhsT=mw_sb[32 * rg:32 * rg + 16, jbq, :],
                rhs=c["e16r"][32 * rg:32 * rg + 16, :],
                start=True, stop=False,
                skip_group_check=True, tile_position=(32 * rg, 0))
            for kb in range(2):
                nc.tensor.matmul(
                    sc_ps[:, rr, kb * S:(kb + 1) * S],
                    lhsT=qk_sb[32 * rg:32 * rg + 32, 2 + jbq,
                               kb * 128:(kb + 1) * 128],
                    rhs=qk_sb[32 * rg:32 * rg + 32, jbq, :],
                    start=False, stop=(kb == 1),
                    skip_group_check=True, tile_position=(32 * rg, 0))
        nc.scalar.activation(
            out=h["expT"][:, jbq, 2 * rgp:2 * rgp + 2, :], in_=sc_ps,
            func=mybir.ActivationFunctionType.Exp, scale=SCALE)

    # -- PV matmuls for one jbq half; Z for both (in stZPV2) ------------
    def stPV(self, i, jbq):
        nc, c, h = self.nc, self.c, self.h[i]
        expT = h["expT"]
        if jbq == 0:
            h["at"] = self.pat.tile([128, 2, S], F32, tag="at", name="at")
        at = h["at"]
        for rg in range(4):
            hh = 4 * jbq + rg
            for kb in range(2):
                nc.tensor.matmul(at[32 * rg:32 * rg + 32, jbq, :],
                                 lhsT=h["v"][:, kb, 32 * hh:32 * hh + 32],
                                 rhs=expT[:, jbq, rg, kb * S:(kb + 1) * S],
                                 start=(jbq == 0 and kb == 0),
                                 stop=(jbq == 1 and kb == 1),
                                 skip_group_check=True,
                                 tile_position=(0, 32 * rg))

    def stZPV2(self, i):
        nc, c, h = self.nc, self.c, self.h[i]
        expT = h["expT"]
        h["zp"] = self.qz.tile([128, 2, S], F32, tag="qz", name="zp")
        zp = h["zp"]
        for jbq in range(2):
            for rg in range(4):
                for kb in range(2):
                    nc.tensor.matmul(
                        zp[32 * rg:32 * rg + 32, jbq, :],
                        lhsT=c["ones32"],
                        rhs=expT[:, jbq, rg, kb * S:(kb + 1) * S],
                        start=(jbq == 0 and kb == 0),
                        stop=(jbq == 1 and kb == 1),
                        skip_group_check=True,
                        tile_position=(0, 32 * rg))
        self.stPV(i, 1)

    # -- E tail: reciprocal + normalize ---------------------------------
    def stNorm(self, i):
        nc, h = self.nc, self.h[i]
        AL = mybir.AluOpType
        rf_sb = self.mp.tile([128, 2, S], F32, tag="rf", name="rf_sb")
        nc.vector.reciprocal_approx_fast(out=rf_sb, in_=h["zp"])
        atn_sb = self.mp.tile([128, 2, S], BF16, tag="atn", name="atn_sb")
        nc.vector.tensor_tensor(out=atn_sb, in0=h["at"], in1=rf_sb,
                                op=AL.mult)
        h["atn"] = atn_sb

    # -- F: out projection + store --------------------------------------
    def stF(self, i):
        nc, c, h = self.nc, self.c, self.h[i]
        b, t = self.tiles[i]
        atn_sb = h["atn"]
        out_sb = self.mp.tile([128, 2, C], F32, tag="out", name="out_sb")
        po = self.vp.tile([128, 2, C], F32, tag="vp", name="po")
        for sb_ in range(2):
            for cc in range(2):
                nc.tensor.matmul(
                    po[:, sb_, :],
                    lhsT=atn_sb[:, cc, sb_ * 128:(sb_ + 1) * 128],
                    rhs=c["wp"][:, cc, :],
                    start=(sb_ == 0 and cc == 0),
                    stop=(not self.has_bf and sb_ == 1 and cc == 1))
            if self.has_bf:
                nc.tensor.matmul(po[:, sb_, :], lhsT=c["onesr"],
                                 rhs=c["bf"], start=False, stop=(sb_ == 1))
        nc.vector.tensor_copy(out=out_sb, in_=po)
        nc.sync.dma_start(out=c["out_d"][b, t].rearrange("s p c -> p s c"),
                          in_=out_sb)
        self.h[i] = {}  # drop handles


def _build_nc(has_bqk, has_bf, nt=T):
    nc = bacc.Bacc("TRN2", target_bir_lowering=False, debug=False)

    xt_d = nc.dram_tensor("xt", [BPC, nt, C, S], BF16, kind="ExternalInput")
    mw_d = nc.dram_tensor("mw16", [BPC, nt, 128, 2, 128], BF16,
                          kind="ExternalInput")
    wqk_d = nc.dram_tensor("wqk_bf", [C, 2 * C], BF16, kind="ExternalInput")
    wv_d = nc.dram_tensor("wv_bf", [C, C], BF16, kind="ExternalInput")
    wp_d = nc.dram_tensor("wproj_bf", [C, C], BF16, kind="ExternalInput")
    e8_d = nc.dram_tensor("e16r", [128, 2 * S], BF16, kind="ExternalInput")
    bqk_d = nc.dram_tensor("bqk_cols", [128, 4], F32, kind="ExternalInput")
    bf_d = nc.dram_tensor("bfinal_row", [1, C], BF16, kind="ExternalInput")
    out_d = nc.dram_tensor("out", [BPC, nt, 2, 128, C], F32,
                           kind="ExternalOutput")

    with tile.TileContext(nc) as tc:
        with (
            tc.tile_pool(name="wpool", bufs=1) as wp,
            tc.tile_pool(name="xpool", bufs=6) as xp,
            tc.tile_pool(name="mid", bufs=4) as mp,
            tc.tile_pool(name="exps", bufs=4) as ep,
            tc.tile_pool(name="qz", bufs=2, space="PSUM") as qz,
            tc.tile_pool(name="vp", bufs=1, space="PSUM") as vp,
            tc.tile_pool(name="at", bufs=1, space="PSUM") as pat,
            tc.tile_pool(name="sc", bufs=2, space="PSUM") as psc,
        ):
            consts = {"xt_d": xt_d, "mw_d": mw_d, "out_d": out_d}
            wqk_sb = wp.tile([128, 2, 2 * C], BF16)
            nc.sync.dma_start(out=wqk_sb,
                              in_=wqk_d.ap().rearrange("(cc p) j -> p cc j",
                                                       p=128))
            consts["wqk"] = wqk_sb
            wv_sb = wp.tile([128, 2, C], BF16)
            nc.sync.dma_start(out=wv_sb,
                              in_=wv_d.ap().rearrange("(cc p) j -> p cc j",
                                                      p=128))
            consts["wv"] = wv_sb
            wp_sb = wp.tile([128, 2, C], BF16)
            nc.sync.dma_start(out=wp_sb,
                              in_=wp_d.ap().rearrange("(cc p) j -> p cc j",
                                                      p=128))
            consts["wp"] = wp_sb
            e16_sb = wp.tile([128, 2 * S], BF16)
            nc.sync.dma_start(out=e16_sb, in_=e8_d.ap())
            consts["e16r"] = e16_sb
            ones32_sb = wp.tile([128, 32], BF16)
            nc.vector.memset(ones32_sb, 1.0)
            consts["ones32"] = ones32_sb
            if has_bqk:
                bqk_sb = wp.tile([128, 4], F32)
                nc.sync.dma_start(out=bqk_sb, in_=bqk_d.ap())
                consts["bqk"] = bqk_sb
            if has_bf:
                bf_sb = wp.tile([1, C], BF16)
                nc.sync.dma_start(out=bf_sb, in_=bf_d.ap())
                consts["bf"] = bf_sb
                onesr_sb = wp.tile([1, 128], BF16)
                nc.vector.memset(onesr_sb, 1.0)
                consts["onesr"] = onesr_sb

            tiles = [(b, t) for b in range(BPC) for t in range(nt)]
            K = _Kern(nc, (xp, mp, ep, qz, vp, pat, psc), consts, tiles,
                      has_bqk, has_bf)
            N = len(tiles)
            # PE warm-up burst: dummy matmuls execute during the initial
            # weight/x DMA window (PE otherwise idle) so the HAM clock
            # gate reaches K=8/8 before the first real tile, instead of
            # running tiles 0-1 at half clock.
            warm_ps = psc.tile([128, 2, 2 * S], F32, tag="sc",
                               name="warm_ps")
            for _w in range(56):
                nc.tensor.matmul(warm_ps[0:32, 0, 0:32],
                                 lhsT=ones32_sb, rhs=ones32_sb,
                                 start=True, stop=True,
                                 skip_group_check=True)
            # Interleaved schedule: while ACT runs exp(i,0) the PE runs
            # Z/PV-jbq1(i-1) and qkv(i+1); while exp(i,1) runs the PE
            # runs Z/PV-jbq0(i) and proj(i-1).
            K.stA(0)
            for i in range(N):
                K.stSC(i, 0, 0)
                if i > 0:
                    K.stZPV2(i - 1)   # PV-jbq1(i-1) + Z(i-1)
                    K.stNorm(i - 1)
                K.stSC(i, 0, 1)
                if i + 1 < N:
                    K.stA(i + 1)
                K.stSC(i, 1, 0)
                K.stPV(i, 0)
                K.stSC(i, 1, 1)
                if i > 0:
                    K.stF(i - 1)
            K.stZPV2(N - 1)
            K.stNorm(N - 1)
            K.stF(N - 1)

    nc.compile()
    return nc


def _host_routing(x4, w_qkv, b_qkv):
    """Top-4 window routing in fp64 on host -> additive mask lhsT layout
    [B, T, 128, 2, 2, 128] bf16 (rows 32*rg + w = mask of q-window w,
    head 4*jbq+rg; free dims (jbq, kb); cols = keys of chunk kb)."""
    xsum = x4.reshape(B, T, NW, WIN, C).sum(3, dtype=np.float64)  # [B,T,NW,C]
    wq = w_qkv[:, :C].astype(np.float64)
    wk = w_qkv[:, C:2 * C].astype(np.float64)
    q_reg = xsum @ wq + WIN * b_qkv[:C].astype(np.float64)
    k_reg = xsum @ wk + WIN * b_qkv[C:2 * C].astype(np.float64)
    q_reg = q_reg.reshape(B, T, NW, NH, D)
    k_reg = k_reg.reshape(B, T, NW, NH, D)
    sim = np.einsum('btnhd,btmhd->bthnm', q_reg, k_reg)  # [B,T,h,NW,NW]
    thr = -np.partition(-sim, TK - 1, axis=-1)[..., TK - 1:TK]
    am = np.where(sim >= thr, 0.0, MASKVAL).astype(np.float32)
    nsel = (am == 0.0).sum(-1)
    if np.any(nsel != TK):  # pragma: no cover - ties are measure-zero
        idx = np.argsort(-sim, axis=-1, kind='stable')[..., :TK]
        am = np.full(sim.shape, MASKVAL, np.float32)
        np.put_along_axis(am, idx, 0.0, axis=-1)
    amx = np.repeat(am, WIN, axis=-1)  # [B,T,h,qw,256]
    mw16 = np.zeros((B, T, 128, 2, 128), np.float32)
    for jbq in range(2):
        for rg in range(4):
            hh = 4 * jbq + rg
            for kb in range(2):
                r0 = 32 * rg + 8 * kb
                mw16[:, :, r0:r0 + 8, jbq, :] = \
                    amx[:, :, hh, :, kb * 128:(kb + 1) * 128]
    return mw16.astype(ml_dtypes.bfloat16)


def _make_e16r():
    e = np.zeros((128, 2 * S), ml_dtypes.bfloat16)
    q = np.arange(S) // WIN  # query window of column q
    for rg in range(4):
        for kb in range(2):
            for w in range(NW):
                e[32 * rg + 8 * kb + w, kb * S + np.arange(S)[q == w]] = 1.0
    return e


def _host_prep(x, w_qkv, b_qkv, w_proj, b_proj):
    bf16 = ml_dtypes.bfloat16
    x4 = x.reshape(B, T, S, C)
    xt = np.ascontiguousarray(x4.transpose(0, 1, 3, 2)).astype(bf16)
    mw16 = _host_routing(x4, w_qkv, b_qkv)

    bfinal = b_qkv[2 * C:] @ w_proj + b_proj
    shared = {
        "wqk_bf": np.ascontiguousarray(w_qkv[:, :2 * C]).astype(bf16),
        "wv_bf": np.ascontiguousarray(w_qkv[:, 2 * C:]).astype(bf16),
        "wproj_bf": w_proj.astype(bf16),
        "e16r": _make_e16r(),
        "bqk_cols": np.ascontiguousarray(
            b_qkv[:2 * C].reshape(4, 128).T).astype(np.float32),
        "bfinal_row": bfinal.reshape(1, C).astype(bf16),
    }
    in_maps = []
    for core in range(NCORES):
        b0 = core * BPC
        m = dict(shared)
        m["xt"] = np.ascontiguousarray(xt[b0:b0 + BPC])
        m["mw16"] = np.ascontiguousarray(mw16[b0:b0 + BPC])
        in_maps.append(m)
    return in_maps


def kernel(x, w_qkv, b_qkv, w_proj, b_proj, **_unused_scalars):
    x = np.asarray(x, dtype=np.float32)
    w_qkv = np.asarray(w_qkv, dtype=np.float32)
    b_qkv = np.asarray(b_qkv, dtype=np.float32)
    w_proj = np.asarray(w_proj, dtype=np.float32)
    b_proj = np.asarray(b_proj, dtype=np.float32)

    has_bqk = bool(np.any(b_qkv[:2 * C]))
    bfinal = b_qkv[2 * C:] @ w_proj + b_proj
    has_bf = bool(np.any(bfinal))
    key = ("nc", has_bqk, has_bf)
    if key not in _CACHE:
        _CACHE[key] = _build_nc(has_bqk, has_bf)
        _CACHE["nc"] = _CACHE[key]
    nc = _CACHE[key]

    in_maps = _host_prep(x, w_qkv, b_qkv, w_proj, b_proj)
    res = run_bass_kernel_spmd(nc, in_maps, core_ids=list(range(NCORES)))

    out = np.empty((B, T, 2, 128, C), np.float32)
    for core in range(NCORES):
        out[core * BPC:(core + 1) * BPC] = res.results[core]["out"]
    # [B, T, sb, p, C] -> [B, T*S, C]
    return out.reshape(B, T * S, C)


# revision 27
# speedup vs baseline: 1.2144x; 1.0063x over previous
"""BiLevelRoutingAttention Trainium2 kernel (v5).

Strategy (8 NeuronCores, data-parallel over batch: 2 batches/core, 32 (b,t)
tiles per core):
  - Host: transpose x to feature-major bf16; ROUTING ON HOST (fp64 window
    sums -> region features -> sim -> top-4 -> additive window mask), mask
    uploaded pre-expanded as [8,128] bf16 matmul lhsT slices that a matmul
    against static one-hot e8r rows expands onto the scores inside PSUM.
  - Device, per (b,t) tile, all layouts feature-major ("T-layout"):
      qT/kT = W^T x^T (bf16 matmuls, fp32 PSUM), V token-major.
      scoresT + mask accumulated in PSUM, exp on ACT (scale folded),
      Z via ones[128,32]-matmuls -> Z broadcast in PSUM (no DRAM bounce),
      reciprocal_approx_fast (DVE), PV col-packed, normalize, out
      projection, store fp32.
  - Emission is a fine-grained software pipeline: per-engine queues are
    in-order, so stages of adjacent tiles are interleaved such that the
    PE always has data-ready matmuls queued while ACT runs exp, and no
    engine's queue head ever waits on work emitted later (deadlock-free
    by construction; PSUM bank sharing pairs chosen to keep all pool
    reuse waits pointing backwards in emission order).
"""

import sys

sys.path.insert(0, "/opt/trn_rl_repo")

import numpy as np
import ml_dtypes

import concourse.bass as bass
import concourse.bacc as bacc
import concourse.mybir as mybir
import concourse.tile as tile
from concourse.bass_utils import run_bass_kernel_spmd

BF16 = mybir.dt.bfloat16
F32 = mybir.dt.float32

NCORES = 8
B, T, S, C = 16, 16, 256, 256
NW, WIN, NH, D, TK = 8, 32, 8, 32, 4
BPC = B // NCORES  # batches per core
SCALE = float(D) ** -0.5
MASKVAL = -1e9

_CACHE = {}


class _Kern:
    """Stage emitters for one (b, t) tile; handles live on self.h[i]."""

    def __init__(self, nc, pools, consts, tiles, has_bqk, has_bf):
        self.nc = nc
        (self.xp, self.mp, self.ep, self.qz, self.vp, self.pat,
         self.psc) = pools
        self.c = consts
        self.tiles = tiles
        self.h = [dict() for _ in tiles]
        self.has_bqk = has_bqk
        self.has_bf = has_bf

    # -- A: loads + qkv projections + PSUM->SBUF casts ------------------
    def stA(self, i):
        nc, c, h = self.nc, self.c, self.h[i]
        b, t = self.tiles[i]
        AL = mybir.AluOpType
        xt_sb = self.xp.tile([128, 2, S], BF16, tag="xt", name="xt_sb")
        nc.sync.dma_start(
            out=xt_sb,
            in_=c["xt_d"][b, t].rearrange("(cc p) s -> p cc s", p=128))
        mw_sb = self.xp.tile([128, 2, 128], BF16, tag="mw", name="mw_sb")
        nc.sync.dma_start(
            out=mw_sb[:].rearrange("p a k -> p (a k)"),
            in_=c["mw_d"][b, t].rearrange("p a k -> p (a k)"))
        h["mw"] = mw_sb

        qk_sb = self.mp.tile([128, 4, S], BF16, tag="qk", name="qk_sb")
        for half in range(2):
            qps = self.qz.tile([128, 2, S], F32, tag="qz", name="qps")
            for j in range(2):
                jb = 2 * half + j
                for cc in range(2):
                    nc.tensor.matmul(
                        qps[:, j, :],
                        lhsT=c["wqk"][:, cc, jb * 128:(jb + 1) * 128],
                        rhs=xt_sb[:, cc, :],
                        start=(j == 0 and cc == 0),
                        stop=(j == 1 and cc == 1))
            if self.has_bqk:
                nc.vector.tensor_tensor(
                    out=qk_sb[:, 2 * half:2 * half + 2, :], in0=qps,
                    in1=c["bqk"][:, 2 * half:2 * half + 2].unsqueeze(-1)
                        .to_broadcast([128, 2, S]),
                    op=AL.add)
            else:
                nc.vector.tensor_copy(
                    out=qk_sb[:, 2 * half:2 * half + 2, :], in_=qps)
        h["qk"] = qk_sb

        v_sb = self.mp.tile([128, 2, C], BF16, tag="v", name="v_sb")
        vps = self.vp.tile([128, 2, C], F32, tag="vp", name="vps")
        for sb_ in range(2):
            for cc in range(2):
                nc.tensor.matmul(vps[:, sb_, :],
                                 lhsT=xt_sb[:, cc, sb_ * 128:(sb_ + 1) * 128],
                                 rhs=c["wv"][:, cc, :],
                                 start=(sb_ == 0 and cc == 0),
                                 stop=(sb_ == 1 and cc == 1))
        nc.vector.tensor_copy(out=v_sb, in_=vps)
        h["v"] = v_sb

    # -- scores + mask for one (jbq, rg-pair) unit, then exp ------------
    # 2-bank PSUM tiles, bufs=2: while ACT exps one buffer the PE fills
    # the other; each concurrent row-group owns a full PSUM bank.
    def stSC(self, i, jbq, rgp):
        nc, c, h = self.nc, self.c, self.h[i]
        if jbq == 0 and rgp == 0:
            h["expT"] = self.ep.tile([128, 2, 4, 2 * S], BF16, tag="expT",
                                     name="expT")
        qk_sb, mw_sb = h["qk"], h["mw"]
        sc_ps = self.psc.tile([128, 2, 2 * S], F32, tag="sc", name="sc_ps")
        for rr in range(2):
            rg = 2 * rgp + rr
            nc.tensor.matmul(
                sc_ps[:, rr, :],
                lhsT=mw_sb[32 * rg:32 * rg + 16, jbq, :],
                rhs=c["e16r"][32 * rg:32 * rg + 16, :],
                start=True, stop=False,
                skip_group_check=True, tile_position=(32 * rg, 0))
            for kb in range(2):
                nc.tensor.matmul(
                    sc_ps[:, rr, kb * S:(kb + 1) * S],
                    lhsT=qk_sb[32 * rg:32 * rg + 32, 2 + jbq,
                               kb * 128:(kb + 1) * 128],
                    rhs=qk_sb[32 * rg:32 * rg + 32, jbq, :],
                    start=False, stop=(kb == 1),
                    skip_group_check=True, tile_position=(32 * rg, 0))
        nc.scalar.activation(
            out=h["expT"][:, jbq, 2 * rgp:2 * rgp + 2, :], in_=sc_ps,
            func=mybir.ActivationFunctionType.Exp, scale=SCALE)

    # -- PV matmuls for one jbq half; Z for both (in stZPV2) ------------
    def stPV(self, i, jbq):
        nc, c, h = self.nc, self.c, self.h[i]
        expT = h["expT"]
        if jbq == 0:
            h["at"] = self.pat.tile([128, 2, S], F32, tag="at", name="at")
        at = h["at"]
        for rg in range(4):
            hh = 4 * jbq + rg
            for kb in range(2):
                nc.tensor.matmul(at[32 * rg:32 * rg + 32, jbq, :],
                                 lhsT=h["v"][:, kb, 32 * hh:32 * hh + 32],
                                 rhs=expT[:, jbq, rg, kb * S:(kb + 1) * S],
                                 start=(jbq == 0 and kb == 0),
                                 stop=(jbq == 1 and kb == 1),
                                 skip_group_check=True,
                                 tile_position=(0, 32 * rg))

    def stZPV2(self, i):
        nc, c, h = self.nc, self.c, self.h[i]
        expT = h["expT"]
        h["zp"] = self.qz.tile([128, 2, S], F32, tag="qz", name="zp")
        zp = h["zp"]
        for jbq in range(2):
            for rg in range(4):
                for kb in range(2):
                    nc.tensor.matmul(
                        zp[32 * rg:32 * rg + 32, jbq, :],
                        lhsT=c["ones32"],
                        rhs=expT[:, jbq, rg, kb * S:(kb + 1) * S],
                        start=(jbq == 0 and kb == 0),
                        stop=(jbq == 1 and kb == 1),
                        skip_group_check=True,
                        tile_position=(0, 32 * rg))
        self.stPV(i, 1)

    # -- E tail: reciprocal + normalize ---------------------------------
    def stNorm(self, i):
        nc, h = self.nc, self.h[i]
        AL = mybir.AluOpType
        rf_sb = self.mp.tile([128, 2, S], F32, tag="rf", name="rf_sb")
        nc.vector.reciprocal_approx_fast(out=rf_sb, in_=h["zp"])
        atn_sb = self.mp.tile([128, 2, S], BF16, tag="atn", name="atn_sb")
        nc.vector.tensor_tensor(out=atn_sb, in0=h["at"], in1=rf_sb,
                                op=AL.mult)
        h["atn"] = atn_sb

    # -- F: out projection + store --------------------------------------
    def stF(self, i):
        nc, c, h = self.nc, self.c, self.h[i]
        b, t = self.tiles[i]
        atn_sb = h["atn"]
        out_sb = self.mp.tile([128, 2, C], F32, tag="out", name="out_sb")
        po = self.vp.tile([128, 2, C], F32, tag="vp", name="po")
        for sb_ in range(2):
            for cc in range(2):
                nc.tensor.matmul(
                    po[:, sb_, :],
                    lhsT=atn_sb[:, cc, sb_ * 128:(sb_ + 1) * 128],
                    rhs=c["wp"][:, cc, :],
                    start=(sb_ == 0 and cc == 0),
                    stop=(not self.has_bf and sb_ == 1 and cc == 1))
            if self.has_bf:
                nc.tensor.matmul(po[:, sb_, :], lhsT=c["onesr"],
                                 rhs=c["bf"], start=False, stop=(sb_ == 1))
        nc.vector.tensor_copy(out=out_sb, in_=po)
        nc.sync.dma_start(out=c["out_d"][b, t].rearrange("s p c -> p s c"),
                          in_=out_sb)
        self.h[i] = {}  # drop handles


def _build_nc(has_bqk, has_bf, nt=T):
    nc = bacc.Bacc("TRN2", target_bir_lowering=False, debug=False)

    xt_d = nc.dram_tensor("xt", [BPC, nt, C, S], BF16, kind="ExternalInput")
    mw_d = nc.dram_tensor("mw16", [BPC, nt, 128, 2, 128], BF16,
                          kind="ExternalInput")
    wqk_d = nc.dram_tensor("wqk_bf", [C, 2 * C], BF16, kind="ExternalInput")
    wv_d = nc.dram_tensor("wv_bf", [C, C], BF16, kind="ExternalInput")
    wp_d = nc.dram_tensor("wproj_bf", [C, C], BF16, kind="ExternalInput")
    e8_d = nc.dram_tensor("e16r", [128, 2 * S], BF16, kind="ExternalInput")
    bqk_d = nc.dram_tensor("bqk_cols", [128, 4], F32, kind="ExternalInput")
    bf_d = nc.dram_tensor("bfinal_row", [1, C], BF16, kind="ExternalInput")
    out_d = nc.dram_tensor("out", [BPC, nt, 2, 128, C], F32,
                           kind="ExternalOutput")

    with tile.TileContext(nc) as tc:
        with (
            tc.tile_pool(name="wpool", bufs=1) as wp,
            tc.tile_pool(name="xpool", bufs=6) as xp,
            tc.tile_pool(name="mid", bufs=4) as mp,
            tc.tile_pool(name="exps", bufs=4) as ep,
            tc.tile_pool(name="qz", bufs=2, space="PSUM") as qz,
            tc.tile_pool(name="vp", bufs=1, space="PSUM") as vp,
            tc.tile_pool(name="at", bufs=1, space="PSUM") as pat,
            tc.tile_pool(name="sc", bufs=2, space="PSUM") as psc,
        ):
            consts = {"xt_d": xt_d, "mw_d": mw_d, "out_d": out_d}
            wqk_sb = wp.tile([128, 2, 2 * C], BF16)
            nc.sync.dma_start(out=wqk_sb,
                              in_=wqk_d.ap().rearrange("(cc p) j -> p cc j",
                                                       p=128))
            consts["wqk"] = wqk_sb
            wv_sb = wp.tile([128, 2, C], BF16)
            nc.sync.dma_start(out=wv_sb,
                              in_=wv_d.ap().rearrange("(cc p) j -> p cc j",
                                                      p=128))
            consts["wv"] = wv_sb
            wp_sb = wp.tile([128, 2, C], BF16)
            nc.sync.dma_start(out=wp_sb,
                              in_=wp_d.ap().rearrange("(cc p) j -> p cc j",
                                                      p=128))
            consts["wp"] = wp_sb
            e16_sb = wp.tile([128, 2 * S], BF16)
            nc.sync.dma_start(out=e16_sb, in_=e8_d.ap())
            consts["e16r"] = e16_sb
            ones32_sb = wp.tile([128, 32], BF16)
            nc.vector.memset(ones32_sb, 1.0)
            consts["ones32"] = ones32_sb
            if has_bqk:
                bqk_sb = wp.tile([128, 4], F32)
                nc.sync.dma_start(out=bqk_sb, in_=bqk_d.ap())
                consts["bqk"] = bqk_sb
            if has_bf:
                bf_sb = wp.tile([1, C], BF16)
                nc.sync.dma_start(out=bf_sb, in_=bf_d.ap())
                consts["bf"] = bf_sb
                onesr_sb = wp.tile([1, 128], BF16)
                nc.vector.memset(onesr_sb, 1.0)
                consts["onesr"] = onesr_sb

            tiles = [(b, t) for b in range(BPC) for t in range(nt)]
            K = _Kern(nc, (xp, mp, ep, qz, vp, pat, psc), consts, tiles,
                      has_bqk, has_bf)
            N = len(tiles)
            # PE warm-up burst: dummy matmuls execute during the initial
            # weight/x DMA window (PE otherwise idle) so the HAM clock
            # gate reaches K=8/8 before the first real tile, instead of
            # running tiles 0-1 at half clock.
            warm_ps = psc.tile([128, 2, 2 * S], F32, tag="sc",
                               name="warm_ps")
            for _w in range(56):
                nc.tensor.matmul(warm_ps[0:32, 0, 0:32],
                                 lhsT=ones32_sb, rhs=ones32_sb,
                                 start=True, stop=True,
                                 skip_group_check=True)
            # Interleaved schedule: while ACT runs exp(i,0) the PE runs
            # Z/PV-jbq1(i-1) and qkv(i+1); while exp(i,1) runs the PE
            # runs Z/PV-jbq0(i) and proj(i-1).
            K.stA(0)
            for i in range(N):
                K.stSC(i, 0, 0)
                if i > 0:
                    K.stZPV2(i - 1)   # PV-jbq1(i-1) + Z(i-1)
                    K.stNorm(i - 1)
                K.stSC(i, 0, 1)
                if i + 1 < N:
                    K.stA(i + 1)
                K.stSC(i, 1, 0)
                K.stPV(i, 0)
                K.stSC(i, 1, 1)
                if i > 0:
                    K.stF(i - 1)
            K.stZPV2(N - 1)
            K.stNorm(N - 1)
            K.stF(N - 1)

    nc.compile()
    return nc


def _host_routing(x4, w_qkv, b_qkv):
    """Top-4 window routing in fp64 on host -> additive mask lhsT layout
    [B, T, 128, 2, 2, 128] bf16 (rows 32*rg + w = mask of q-window w,
    head 4*jbq+rg; free dims (jbq, kb); cols = keys of chunk kb)."""
    xsum = x4.reshape(B, T, NW, WIN, C).sum(3, dtype=np.float64)  # [B,T,NW,C]
    wq = w_qkv[:, :C].astype(np.float64)
    wk = w_qkv[:, C:2 * C].astype(np.float64)
    q_reg = xsum @ wq + WIN * b_qkv[:C].astype(np.float64)
    k_reg = xsum @ wk + WIN * b_qkv[C:2 * C].astype(np.float64)
    q_reg = q_reg.reshape(B, T, NW, NH, D)
    k_reg = k_reg.reshape(B, T, NW, NH, D)
    sim = np.einsum('btnhd,btmhd->bthnm', q_reg, k_reg)  # [B,T,h,NW,NW]
    thr = -np.partition(-sim, TK - 1, axis=-1)[..., TK - 1:TK]
    am = np.where(sim >= thr, 0.0, MASKVAL).astype(np.float32)
    nsel = (am == 0.0).sum(-1)
    if np.any(nsel != TK):  # pragma: no cover - ties are measure-zero
        idx = np.argsort(-sim, axis=-1, kind='stable')[..., :TK]
        am = np.full(sim.shape, MASKVAL, np.float32)
        np.put_along_axis(am, idx, 0.0, axis=-1)
    amx = np.repeat(am, WIN, axis=-1)  # [B,T,h,qw,256]
    mw16 = np.zeros((B, T, 128, 2, 128), np.float32)
    for jbq in range(2):
        for rg in range(4):
            hh = 4 * jbq + rg
            for kb in range(2):
                r0 = 32 * rg + 8 * kb
                mw16[:, :, r0:r0 + 8, jbq, :] = \
                    amx[:, :, hh, :, kb * 128:(kb + 1) * 128]
    return mw16.astype(ml_dtypes.bfloat16)


def _make_e16r():
    e = np.zeros((128, 2 * S), ml_dtypes.bfloat16)
    q = np.arange(S) // WIN  # query window of column q
    for rg in range(4):
        for kb in range(2):
            for w in range(NW):
                e[32 * rg + 8 * kb + w, kb * S + np.arange(S)[q == w]] = 1.0
    return e


def _host_prep(x, w_qkv, b_qkv, w_proj, b_proj):
    bf16 = ml_dtypes.bfloat16
    x4 = x.reshape(B, T, S, C)
    xt = np.ascontiguousarray(x4.transpose(0, 1, 3, 2)).astype(bf16)
    mw16 = _host_routing(x4, w_qkv, b_qkv)

    bfinal = b_qkv[2 * C:] @ w_proj + b_proj
    shared = {
        "wqk_bf": np.ascontiguousarray(w_qkv[:, :2 * C]).astype(bf16),
        "wv_bf": np.ascontiguousarray(w_qkv[:, 2 * C:]).astype(bf16),
        "wproj_bf": w_proj.astype(bf16),
        "e16r": _make_e16r(),
        "bqk_cols": np.ascontiguousarray(
            b_qkv[:2 * C].reshape(4, 128).T).astype(np.float32),
        "bfinal_row": bfinal.reshape(1, C).astype(bf16),
    }
    in_maps = []
    for core in range(NCORES):
        b0 = core * BPC
        m = dict(shared)
        m["xt"] = np.ascontiguousarray(xt[b0:b0 + BPC])
        m["mw16"] = np.ascontiguousarray(mw16[b0:b0 + BPC])
        in_maps.append(m)
    return in_maps


def kernel(x, w_qkv, b_qkv, w_proj, b_proj, **_unused_scalars):
    x = np.asarray(x, dtype=np.float32)
    w_qkv = np.asarray(w_qkv, dtype=np.float32)
    b_qkv = np.asarray(b_qkv, dtype=np.float32)
    w_proj = np.asarray(w_proj, dtype=np.float32)
    b_proj = np.asarray(b_proj, dtype=np.float32)

    has_bqk = bool(np.any(b_qkv[:2 * C]))
    bfinal = b_qkv[2 * C:] @ w_proj + b_proj
    has_bf = bool(np.any(bfinal))
    key = ("nc", has_bqk, has_bf)
    if key not in _CACHE:
        _CACHE[key] = _build_nc(has_bqk, has_bf)
        _CACHE["nc"] = _CACHE[key]
    nc = _CACHE[key]

    in_maps = _host_prep(x, w_qkv, b_qkv, w_proj, b_proj)
    res = run_bass_kernel_spmd(nc, in_maps, core_ids=list(range(NCORES)))

    out = np.empty((B, T, 2, 128, C), np.float32)
    for core in range(NCORES):
        out[core * BPC:(core + 1) * BPC] = res.results[core]["out"]
    # [B, T, sb, p, C] -> [B, T*S, C]
    return out.reshape(B, T * S, C)
